# revision 22
# baseline (speedup 1.0000x reference)
"""GAT (2-layer, 3-head) forward on 8 Trainium2 NeuronCores.

Sharding: nodes split 8 ways; each core owns 12544 padded destination nodes
and all their incoming edges (1D graph partition per the spec hint). A
channel-major node table (h | a_src | ones, 15 ch) is replicated into SBUF
as 4 quarters x 2 copies across the 8 GPSIMD 16-partition groups; per-edge
features stream out via ap_gather with per-group index streams laid out in
dst-canonical order with K=8 slots per (dst, group). All gather/fold index
streams and the a_dst broadcast table are preloaded into SBUF once. Per-edge
softmax weights are computed densely on all 128 partitions (broadcast add +
Lrelu/Exp on ACT), replicated to the h channels with a single 0/1 PE matmul
into PSUM, applied with one DVE multiply, and slot-window-reduced with an
in-place binary tree. Overflow rows are processed first into a small ovbuf
and folded back per chunk with a second tiny gather. Cross-group combine +
denominator replication uses lhsn/lhsd PE matmuls. Three NEFF launches:
(A) table build (x @ W1aug on PE), (B) edge layer 1 + layer-2 table build,
(C) edge layer 2 + head-mean + log_softmax. Tables are all-gathered between
launches through the host.
"""
import sys
import types

sys.path.insert(0, "/opt/trn_rl_repo")
import ml_dtypes
import numpy as np

N_NODES = 100000
IN_DIM = 256
HID = 3
HEADS = 3
NCLS = 3
NEG = 0.2
EPS = 1e-16

NQ = 4
QREAL = 25000
QN = 25088
NPAD = NQ * QN          # 100352
NCORE = 8
CN = NPAD // NCORE      # 12544
K = 8
DCHUNK = 224
NCHUNK = CN // DCHUNK   # 56
RPAD = CN + 2 * DCHUNK  # 12992
GCHUNK = RPAD // DCHUNK  # 58
OVW = RPAD - CN         # 448 overflow cols
SLOTS = RPAD * K
SCHUNK = DCHUNK * K     # 1792
SENT = QREAL
ZCOL = RPAD - 1
CH = 15
BIG_NEG = -30000.0

LAST_STATS = {}


def _install_ntff_hook_module():
    if "antenv.axon_hooks" in sys.modules:
        return
    mod = types.ModuleType("antenv.axon_hooks")
    state = {"hook": None, "tried": False}

    def set_axon_ntff_profile_hook(hook):
        state["hook"] = hook

    def get_axon_ntff_profile_hook():
        if state["hook"] is None and not state["tried"]:
            state["tried"] = True
            try:
                from trn_agent_boot.trn_boot import _ntff_profile_via_ctypes

                state["hook"] = _ntff_profile_via_ctypes("/opt/axon/libaxon_pjrt.so")
            except Exception:
                state["hook"] = None
        return state["hook"]

    mod.set_axon_ntff_profile_hook = set_axon_ntff_profile_hook
    mod.get_axon_ntff_profile_hook = get_axon_ntff_profile_hook
    sys.modules["antenv.axon_hooks"] = mod


_install_ntff_hook_module()

import concourse.bass as bass
import concourse.mybir as mybir
import concourse.tile as tile
from concourse.bass_utils import run_bass_kernel_spmd
from concourse.library_overlay import lower_extended_insts
from concourse import library_config

F32 = mybir.dt.float32
BF16 = mybir.dt.bfloat16
I16 = mybir.dt.int16
ALU = mybir.AluOpType
ACT = mybir.ActivationFunctionType


def _split_wide_waits(nc):
    """Walrus here caps sync-wait commands per instruction; hoist excess waits
    onto preceding same-engine nofuse NOPs (engines execute in order)."""
    for fn in nc.m.functions:
        for bb in fn.blocks:
            new_insts = []
            for inst in bb.instructions:
                keep = 0 if isinstance(inst, mybir.InstDrain) else 1
                si = inst.sync_info
                if si is not None and si.on_wait is not None and len(si.on_wait) > keep:
                    waits = list(si.on_wait)
                    head, rest = (waits[:-keep], waits[-keep:]) if keep else (waits, [])
                    while head:
                        chunk, head = head[:1], head[1:]
                        nop = mybir.InstNoOp(name=f"I-{nc.next_id()}", ins=[], outs=[])
                        nop.engine = inst.engine
                        nop.bass_nofuse = True
                        nop.sync_info = mybir.SyncInfo(on_wait=chunk, on_update=[])
                        nc.register_instruction(nop, overwrite=True)
                        new_insts.append(nop)
                    inst.sync_info = mybir.SyncInfo(
                        on_wait=rest, on_update=list(si.on_update or [])
                    )
                new_insts.append(inst)
            bb.instructions.clear()
            for i in new_insts:
                bb.add_instruction(i)


def _run(nc, in_maps, trace=False):
    lower_extended_insts(nc)
    _split_wide_waits(nc)
    return run_bass_kernel_spmd(nc, in_maps, core_ids=list(range(NCORE)), trace=trace)


# ---------------------------------------------------------------- launch A
def _build_phase_a():
    nc = bass.Bass("TRN2")
    xT_d = nc.dram_tensor("xT", [IN_DIM, CN], F32, kind="ExternalInput")
    w1_d = nc.dram_tensor("w1", [IN_DIM, HEADS * HID], F32, kind="ExternalInput")
    w1t_d = nc.dram_tensor("w1t", [HEADS * HID, IN_DIM], F32, kind="ExternalInput")
    attw1_d = nc.dram_tensor("attw1", [HEADS * HID, 6], F32, kind="ExternalInput")
    tab_d = nc.dram_tensor("tab", [CH, CN], F32, kind="ExternalOutput")

    with tile.TileContext(nc) as tc:
        with (
            tc.tile_pool(name="const", bufs=1) as cpool,
            tc.tile_pool(name="io", bufs=3) as iopool,
            tc.tile_pool(name="ps", bufs=2, space="PSUM") as pspool,
        ):
            w1aug = cpool.tile([128, 2 * CH], F32)
            w1t = cpool.tile([HEADS * HID, IN_DIM], F32)
            attw1 = cpool.tile([HEADS * HID, 6], F32)
            nc.sync.dma_start(w1t[:], w1t_d[:])
            nc.sync.dma_start(attw1[:], attw1_d[:])
            for k in range(2):
                nc.sync.dma_start(
                    w1aug[:, CH * k:CH * k + 9], w1_d[128 * k:128 * (k + 1), :]
                )
                vps = pspool.tile([128, 6], F32, tag="vps")
                nc.tensor.matmul(
                    out=vps[:],
                    lhsT=w1t[:, 128 * k:128 * (k + 1)],
                    rhs=attw1[:],
                    start=True,
                    stop=True,
                )
                nc.vector.tensor_copy(out=w1aug[:, CH * k + 9:CH * k + 15], in_=vps[:])
            for c in range(NCHUNK):
                cols = slice(DCHUNK * c, DCHUNK * (c + 1))
                ps = pspool.tile([CH, DCHUNK], F32, tag="ps")
                for k in range(2):
                    xc = iopool.tile([128, DCHUNK], F32, tag="xc")
                    nc.sync.dma_start(xc[:], xT_d[128 * k:128 * (k + 1), cols])
                    nc.tensor.matmul(
                        out=ps[:],
                        lhsT=w1aug[:, CH * k:CH * (k + 1)],
                        rhs=xc[:],
                        start=(k == 0),
                        stop=(k == 1),
                    )
                ob = iopool.tile([CH, DCHUNK], F32, tag="ob")
                nc.vector.tensor_copy(out=ob[:], in_=ps[:])
                nc.sync.dma_start(tab_d[:, cols], ob[:])
    return nc


# ---------------------------------------------------------------- launch B/C
def _build_edge(final):
    nc = bass.Bass("TRN2")
    tab_d = nc.dram_tensor("tabf", [CH, NPAD], F32, kind="ExternalInput")
    idx_d = nc.dram_tensor("idxs", [128, SLOTS // 16], I16, kind="ExternalInput")
    ov_d = nc.dram_tensor(
        "ovidx", [128, NCHUNK * DCHUNK // 16], I16, kind="ExternalInput"
    )
    adrep_d = nc.dram_tensor("adrep", [128, RPAD], BF16, kind="ExternalInput")
    w9map_d = nc.dram_tensor("w9map", [128, 128], F32, kind="ExternalInput")
    lhsn_d = nc.dram_tensor("lhsn", [128, 9], F32, kind="ExternalInput")
    lhsd_d = nc.dram_tensor("lhsd", [128, 9], F32, kind="ExternalInput")
    bias_d = nc.dram_tensor("biasv", [9, 1], F32, kind="ExternalInput")
    if final:
        meanw_d = nc.dram_tensor("meanw", [9, NCLS], F32, kind="ExternalInput")
        ones3_d = nc.dram_tensor("ones3", [NCLS, 1], F32, kind="ExternalInput")
        ones1_d = nc.dram_tensor("ones1", [1, NCLS], F32, kind="ExternalInput")
        out_d = nc.dram_tensor("outp", [NCLS, CN], F32, kind="ExternalOutput")
    else:
        w2aug_d = nc.dram_tensor("w2aug", [9, CH], F32, kind="ExternalInput")
        tab2_d = nc.dram_tensor("tab2", [CH, CN], F32, kind="ExternalOutput")

    with tile.TileContext(nc) as tc:
        with (
            tc.tile_pool(name="big", bufs=1) as bigpool,
            tc.tile_pool(name="gp", bufs=2) as gpool,
            tc.tile_pool(name="wp", bufs=2) as wpool,
            tc.tile_pool(name="pp", bufs=2) as ppool,
            tc.tile_pool(name="sm", bufs=2) as smpool,
            tc.tile_pool(name="pw", bufs=1, space="PSUM") as pwpool,
            tc.tile_pool(name="pn", bufs=2, space="PSUM") as pnpool,
        ):
            table = bigpool.tile([128, QN], F32)
            idxpre = bigpool.tile([128, SLOTS // 16], I16)
            ovpre = bigpool.tile([128, NCHUNK * DCHUNK // 16], I16)
            adrep = bigpool.tile([128, RPAD], BF16)
            w9map = bigpool.tile([128, 128], F32)
            lhsn = bigpool.tile([128, 9], F32)
            lhsd = bigpool.tile([128, 9], F32)
            biasv = bigpool.tile([9, 1], F32)
            ovbuf = bigpool.tile([128, OVW], F32)
            for g in range(8):
                q = g % 4
                nc.sync.dma_start(
                    table[16 * g:16 * g + CH, :], tab_d[:, QN * q:QN * (q + 1)]
                )
                # channel 15 of each group is never produced by the table DMA;
                # fill it with the ones row so gathers can't read uninit SBUF.
                nc.sync.dma_start(
                    table[16 * g + CH:16 * g + 16, :], tab_d[CH - 1:CH, QN * q:QN * (q + 1)]
                )
            nc.sync.dma_start(idxpre[:], idx_d[:])
            nc.sync.dma_start(ovpre[:], ov_d[:])
            nc.sync.dma_start(adrep[:], adrep_d[:])
            nc.sync.dma_start(w9map[:], w9map_d[:])
            nc.sync.dma_start(lhsn[:], lhsn_d[:])
            nc.sync.dma_start(lhsd[:], lhsd_d[:])
            nc.sync.dma_start(biasv[:], bias_d[:])
            if final:
                meanw = bigpool.tile([9, NCLS], F32)
                ones3 = bigpool.tile([NCLS, 1], F32)
                ones1 = bigpool.tile([1, NCLS], F32)
                nc.sync.dma_start(meanw[:], meanw_d[:])
                nc.sync.dma_start(ones3[:], ones3_d[:])
                nc.sync.dma_start(ones1[:], ones1_d[:])
            else:
                w2aug = bigpool.tile([9, CH], F32)
                nc.sync.dma_start(w2aug[:], w2aug_d[:])

            tab_in = table[:].rearrange("p (n d) -> p n d", d=1)
            ov_in = ovbuf[:].rearrange("p (n d) -> p n d", d=1)
            nc.gpsimd.load_library(library_config.ap_gather)

            def do_chunk(c):
                """Gather + per-slot weights + weighted slot reduce for grid
                chunk c; returns the [128, DCHUNK] reduced partials tile."""
                scol = slice(SCHUNK // 16 * c, SCHUNK // 16 * (c + 1))
                dcol = slice(DCHUNK * c, DCHUNK * (c + 1))
                g_t = gpool.tile([128, SCHUNK], F32, tag="g")
                nc.gpsimd.ap_gather(
                    out_ap=g_t[:].rearrange("p (n d) -> p n d", d=1),
                    in_ap=tab_in,
                    idxs_ap=idxpre[:, scol],
                    channels=128,
                    num_elems=QN,
                    d=1,
                    num_idxs=SCHUNK,
                )
                # per-slot logits on all 128 rows (only 16g+9..11 meaningful)
                wadd = wpool.tile([128, SCHUNK], F32, tag="w")
                wadd3 = wadd[:].rearrange("p (n j) -> p n j", j=K)
                g3 = g_t[:].rearrange("p (n j) -> p n j", j=K)
                nc.vector.tensor_tensor(
                    out=wadd3, in0=g3,
                    in1=adrep[:, dcol].to_broadcast([128, DCHUNK, K]), op=ALU.add,
                )
                nc.scalar.activation(
                    out=wadd[:], in_=wadd[:], func=ACT.Lrelu, alpha=NEG
                )
                nc.scalar.activation(out=wadd[:], in_=wadd[:], func=ACT.Exp)
                # replicate head weights to channel rows with one 0/1 matmul
                w9ps = pwpool.tile([128, SCHUNK], F32, tag="w9")
                for s0 in range(0, SCHUNK, 512):
                    s1 = min(s0 + 512, SCHUNK)
                    nc.tensor.matmul(
                        out=w9ps[:, s0:s1], lhsT=w9map[:], rhs=wadd[:, s0:s1],
                        start=True, stop=True,
                    )
                nc.vector.tensor_tensor(
                    out=g_t[:], in0=g_t[:], in1=w9ps[:], op=ALU.mult
                )
                # in-place binary-tree reduce over the K=8 slot window
                nc.vector.tensor_tensor(
                    out=g3[:, :, 0:4], in0=g3[:, :, 0:4], in1=g3[:, :, 4:8],
                    op=ALU.add,
                )
                nc.vector.tensor_tensor(
                    out=g3[:, :, 0:2], in0=g3[:, :, 0:2], in1=g3[:, :, 2:4],
                    op=ALU.add,
                )
                part = ppool.tile([128, DCHUNK], F32, tag="part")
                nc.vector.tensor_tensor(
                    out=part[:].rearrange("p (n j) -> p n j", j=1),
                    in0=g3[:, :, 0:1], in1=g3[:, :, 1:2], op=ALU.add,
                )
                return part

            # overflow grid chunks first -> ovbuf
            for i, c in enumerate(range(NCHUNK, GCHUNK)):
                part = do_chunk(c)
                nc.vector.tensor_copy(
                    out=ovbuf[:, DCHUNK * i:DCHUNK * (i + 1)], in_=part[:]
                )

            # main chunks: fold overflow + combine + per-node math
            for c in range(NCHUNK):
                dcol = slice(DCHUNK * c, DCHUNK * (c + 1))
                part = do_chunk(c)
                fold = ppool.tile([128, DCHUNK], F32, tag="fold")
                nc.gpsimd.ap_gather(
                    out_ap=fold[:].rearrange("p (n d) -> p n d", d=1),
                    in_ap=ov_in,
                    idxs_ap=ovpre[:, DCHUNK // 16 * c:DCHUNK // 16 * (c + 1)],
                    channels=128,
                    num_elems=OVW,
                    d=1,
                    num_idxs=DCHUNK,
                )
                ndn = pnpool.tile([9, DCHUNK], F32, tag="ndn")
                ndd = pnpool.tile([9, DCHUNK], F32, tag="ndd")
                nc.tensor.matmul(
                    out=ndn[:], lhsT=lhsn[:], rhs=part[:], start=True, stop=False
                )
                nc.tensor.matmul(
                    out=ndn[:], lhsT=lhsn[:], rhs=fold[:], start=False, stop=True
                )
                nc.tensor.matmul(
                    out=ndd[:], lhsT=lhsd[:], rhs=part[:], start=True, stop=False
                )
                nc.tensor.matmul(
                    out=ndd[:], lhsT=lhsd[:], rhs=fold[:], start=False, stop=True
                )
                rden = smpool.tile([9, DCHUNK], F32, tag="rden")
                nc.vector.tensor_scalar_add(out=rden[:], in0=ndd[:], scalar1=EPS)
                nc.vector.reciprocal(out=rden[:], in_=rden[:])
                hagg = smpool.tile([9, DCHUNK], F32, tag="hagg")
                nc.vector.tensor_tensor(
                    out=hagg[:], in0=ndn[:], in1=rden[:], op=ALU.mult
                )
                nc.vector.tensor_scalar_add(
                    out=hagg[:], in0=hagg[:], scalar1=biasv[:]
                )
                if not final:
                    # elu = relu(x) + exp(min(x,0)) - 1
                    t1 = smpool.tile([9, DCHUNK], F32, tag="t1")
                    nc.vector.tensor_scalar_min(out=t1[:], in0=hagg[:], scalar1=0.0)
                    nc.scalar.activation(out=t1[:], in_=t1[:], func=ACT.Exp)
                    nc.vector.tensor_scalar_max(out=hagg[:], in0=hagg[:], scalar1=0.0)
                    nc.vector.tensor_tensor(
                        out=hagg[:], in0=hagg[:], in1=t1[:], op=ALU.add
                    )
                    nc.vector.tensor_scalar_add(out=hagg[:], in0=hagg[:], scalar1=-1.0)
                    t2 = pnpool.tile([CH, DCHUNK], F32, tag="ndn")
                    nc.tensor.matmul(
                        out=t2[:], lhsT=w2aug[:], rhs=hagg[:], start=True, stop=True
                    )
                    t2sb = smpool.tile([CH, DCHUNK], F32, tag="t2sb")
                    nc.vector.tensor_copy(out=t2sb[:], in_=t2[:])
                    nc.sync.dma_start(tab2_d[:, dcol], t2sb[:])
                else:
                    zps = pnpool.tile([NCLS, DCHUNK], F32, tag="ndn")
                    nc.tensor.matmul(
                        out=zps[:], lhsT=meanw[:], rhs=hagg[:], start=True, stop=True
                    )
                    z = smpool.tile([NCLS, DCHUNK], F32, tag="z")
                    nc.vector.tensor_scalar_add(
                        out=z[:], in0=zps[:], scalar1=biasv[0:NCLS, :]
                    )
                    ez = smpool.tile([NCLS, DCHUNK], F32, tag="ez")
                    nc.scalar.activation(out=ez[:], in_=z[:], func=ACT.Exp)
                    sps = pnpool.tile([1, DCHUNK], F32, tag="ndd")
                    nc.tensor.matmul(
                        out=sps[:], lhsT=ones3[:], rhs=ez[:], start=True, stop=True
                    )
                    s = smpool.tile([1, DCHUNK], F32, tag="s")
                    nc.scalar.activation(out=s[:], in_=sps[:], func=ACT.Ln)
                    l3ps = pnpool.tile([NCLS, DCHUNK], F32, tag="ndd")
                    nc.tensor.matmul(
                        out=l3ps[:], lhsT=ones1[:], rhs=s[:], start=True, stop=True
                    )
                    l3 = smpool.tile([NCLS, DCHUNK], F32, tag="l3")
                    nc.vector.tensor_copy(out=l3[:], in_=l3ps[:])
                    zm = smpool.tile([NCLS, DCHUNK], F32, tag="zm")
                    nc.vector.tensor_tensor(
                        out=zm[:], in0=z[:], in1=l3[:], op=ALU.subtract
                    )
                    nc.sync.dma_start(out_d[:, dcol], zm[:])
    return nc


# ---------------------------------------------------------------- host side
def _relabel(n):
    q = n // QREAL
    return q * QN + n % QREAL


def _wrap_chunked(stream, chunk):
    """[G, S] streams -> [16G, S//16] ap_gather idx layout, wrapped per chunk."""
    g, s = stream.shape
    nch = s // chunk
    w = stream.reshape(g, nch, chunk // 16, 16)
    w = w.transpose(0, 3, 1, 2)
    return np.ascontiguousarray(w.reshape(g * 16, s // 16))


def _pack_edges(src, dst):
    srcN = _relabel(src.astype(np.int64))
    dstN = _relabel(dst.astype(np.int64))
    core = dstN // CN
    dloc = dstN % CN
    q = srcN // QN
    sloc = (srcN % QN).astype(np.int16)

    key = (core * CN + dloc) * 4 + q
    order = np.argsort(key, kind="stable")
    ks = key[order]
    grp_start = np.r_[0, np.flatnonzero(np.diff(ks)) + 1]
    sizes = np.diff(np.r_[grp_start, len(ks)])
    rank = np.arange(len(ks)) - np.repeat(grp_start, sizes)

    co, dl, qo, sl = core[order], dloc[order], q[order], sloc[order]

    streams = np.full((NCORE, 8, SLOTS), SENT, dtype=np.int16)
    ovidx = np.full((NCORE, CN), ZCOL, dtype=np.int16)
    ovdst = [[] for _ in range(NCORE)]

    main = rank < 16
    gmain = qo[main] + 4 * (rank[main] & 1)
    pos = dl[main] * K + (rank[main] >> 1)
    streams[co[main], gmain, pos] = sl[main]

    for i in np.flatnonzero(~main):
        c, d, qq, s_, r = co[i], dl[i], qo[i], sl[i], rank[i]
        if ovidx[c, d] == ZCOL:
            row = CN + len(ovdst[c])
            assert row < RPAD - 1, "overflow area exhausted"
            ovidx[c, d] = row
            ovdst[c].append(int(d))
        rr = r - 16
        assert rr < 16, "per-(dst,quarter) capacity exceeded"
        g = qq + 4 * (rr & 1)
        streams[c, g, int(ovidx[c, d]) * K + (rr >> 1)] = s_
    return streams, ovidx, ovdst


def kernel(x, edge_index, W1, att_src1, att_dst1, b1, W2, att_src2, att_dst2, b2):
    import os as _os
    import time as _time

    x = np.asarray(x, np.float32)
    W1 = np.asarray(W1, np.float32)
    W2 = np.asarray(W2, np.float32)
    b1v = np.asarray(b1, np.float32)
    b2v = np.asarray(b2, np.float32)

    loops = np.arange(N_NODES, dtype=np.int64)
    src = np.concatenate([np.asarray(edge_index[0], np.int64), loops])
    dst = np.concatenate([np.asarray(edge_index[1], np.int64), loops])
    streams, ovidx, ovdst = _pack_edges(src, dst)

    xP = np.zeros((NPAD, IN_DIM), np.float32)
    xP[_relabel(np.arange(N_NODES))] = x
    xT = np.ascontiguousarray(xP.T)

    def attw(att_s, att_d):
        a = np.zeros((HEADS * HID, 6), np.float32)
        for h in range(HEADS):
            for cc in range(3):
                a[3 * h + cc, h] = np.asarray(att_s, np.float32)[h, cc]
                a[3 * h + cc, 3 + h] = np.asarray(att_d, np.float32)[h, cc]
        return a

    attw1 = attw(att_src1, att_dst1)
    attw2 = attw(att_src2, att_dst2)

    # w9map: psum row 16g+3h+c (h channels) and 16g+12+h (ones channels)
    # both take the exp'd logit living on row 16g+9+h
    w9map = np.zeros((128, 128), np.float32)
    lhsn = np.zeros((128, 9), np.float32)
    lhsd = np.zeros((128, 9), np.float32)
    for g in range(8):
        for h in range(HEADS):
            for cc in range(3):
                w9map[16 * g + 9 + h, 16 * g + 3 * h + cc] = 1.0
                lhsd[16 * g + 12 + h, 3 * h + cc] = 1.0
            w9map[16 * g + 9 + h, 16 * g + 12 + h] = 1.0
        for j in range(9):
            lhsn[16 * g + j, j] = 1.0

    meanw = np.zeros((9, NCLS), np.float32)
    for h in range(HEADS):
        for cc in range(NCLS):
            meanw[3 * h + cc, cc] = 1.0 / 3.0
    ones3 = np.ones((NCLS, 1), np.float32)
    ones1 = np.ones((1, NCLS), np.float32)
    b1m = b1v.reshape(9, 1).copy()
    b2m = np.zeros((9, 1), np.float32)
    b2m[:NCLS, 0] = b2v
    w2aug = np.concatenate([W2, W2 @ attw2], axis=1).astype(np.float32)

    idx_wr = np.stack([_wrap_chunked(streams[c], SCHUNK) for c in range(NCORE)])
    # fold indices rebased into ovbuf coords (sentinel ZCOL -> OVW-1, a
    # guaranteed all-sentinel zero column)
    ov_wr = np.stack(
        [
            _wrap_chunked(
                np.repeat((ovidx[c] - CN).reshape(1, CN), 8, axis=0), DCHUNK
            )
            for c in range(NCORE)
        ]
    )

    def make_adrep(tab):
        """[128, RPAD] bf16 per core: rows 16g+9+h = a_dst[h] of the col's
        dst node (incl. overflow rows); all other rows 0."""
        out = []
        for c in range(NCORE):
            ad = np.zeros((3, RPAD), np.float32)
            ad[:, :CN] = tab[12:15, CN * c:CN * (c + 1)]
            for i, d in enumerate(ovdst[c]):
                ad[:, CN + i] = tab[12:15, CN * c + d]
            rep = np.zeros((128, RPAD), np.float32)
            for g in range(8):
                rep[16 * g + 9:16 * g + 12, :] = ad
            out.append(rep.astype(ml_dtypes.bfloat16))
        return out

    trace = bool(int(_os.environ.get("KERNEL_TRACE", "0")))
    stats = {}
    t0 = _time.time()

    ncA = _build_phase_a()
    in_maps = [
        {
            "xT": np.ascontiguousarray(xT[:, CN * c:CN * (c + 1)]),
            "w1": W1,
            "w1t": np.ascontiguousarray(W1.T),
            "attw1": attw1,
        }
        for c in range(NCORE)
    ]
    resA = _run(ncA, in_maps, trace=trace)
    stats["A_ns"] = resA.exec_time_ns
    tab1 = np.concatenate([resA.results[c]["tab"] for c in range(NCORE)], axis=1)
    padmask = np.zeros(NPAD, bool)
    for qq in range(NQ):
        padmask[QN * qq + QREAL:QN * (qq + 1)] = True
    adreps = make_adrep(tab1)
    tab1[9:12, padmask] = BIG_NEG
    tab1[12:15, :] = 1.0

    ncB = _build_edge(final=False)
    in_maps = [
        {
            "tabf": tab1,
            "idxs": idx_wr[c],
            "ovidx": ov_wr[c],
            "adrep": adreps[c],
            "w9map": w9map,
            "lhsn": lhsn,
            "lhsd": lhsd,
            "biasv": b1m,
            "w2aug": w2aug,
        }
        for c in range(NCORE)
    ]
    resB = _run(ncB, in_maps, trace=trace)
    stats["B_ns"] = resB.exec_time_ns
    tab2 = np.concatenate([resB.results[c]["tab2"] for c in range(NCORE)], axis=1)
    adreps = make_adrep(tab2)
    tab2[9:12, padmask] = BIG_NEG
    tab2[12:15, :] = 1.0

    ncC = _build_edge(final=True)
    in_maps = [
        {
            "tabf": tab2,
            "idxs": idx_wr[c],
            "ovidx": ov_wr[c],
            "adrep": adreps[c],
            "w9map": w9map,
            "lhsn": lhsn,
            "lhsd": lhsd,
            "biasv": b2m,
            "meanw": meanw,
            "ones3": ones3,
            "ones1": ones1,
        }
        for c in range(NCORE)
    ]
    resC = _run(ncC, in_maps, trace=trace)
    stats["C_ns"] = resC.exec_time_ns
    outT = np.concatenate([resC.results[c]["outp"] for c in range(NCORE)], axis=1)
    stats["wall_s"] = _time.time() - t0

    out = outT.T[_relabel(np.arange(N_NODES))]
    LAST_STATS.clear()
    LAST_STATS.update(stats)
    return np.ascontiguousarray(out, dtype=np.float32)




# revision 23
# speedup vs baseline: 1.0579x; 1.0579x over previous
"""GAT (2-layer, 3-head) forward on 8 Trainium2 NeuronCores.

Sharding: nodes split 8 ways; each core owns 12544 padded destination nodes
and all their incoming edges (1D graph partition per the spec hint). A
channel-major node table (h | a_src | ones, 15 ch) is replicated into SBUF
as 4 quarters x 2 copies across the 8 GPSIMD 16-partition groups; per-edge
features stream out via ap_gather with per-group index streams laid out in
dst-canonical order with K=8 slots per (dst, group). All gather/fold index
streams and the a_dst broadcast table are preloaded into SBUF once. Per-edge
softmax weights are computed densely on all 128 partitions (broadcast add +
Lrelu/Exp on ACT), replicated to the h channels with a single 0/1 PE matmul
into PSUM, applied with one DVE multiply, and slot-window-reduced with an
in-place binary tree. Overflow rows are processed first into a small ovbuf
and folded back per chunk with a second tiny gather. Cross-group combine +
denominator replication uses lhsn/lhsd PE matmuls. Three NEFF launches:
(A) table build (x @ W1aug on PE), (B) edge layer 1 + layer-2 table build,
(C) edge layer 2 + head-mean + log_softmax. Tables are all-gathered between
launches through the host.
"""
import sys
import types

sys.path.insert(0, "/opt/trn_rl_repo")
import ml_dtypes
import numpy as np

N_NODES = 100000
IN_DIM = 256
HID = 3
HEADS = 3
NCLS = 3
NEG = 0.2
EPS = 1e-16

NQ = 4
QREAL = 25000
QN = 25088
NPAD = NQ * QN          # 100352
NCORE = 8
CN = NPAD // NCORE      # 12544
K = 7
CAP = 2 * K             # main-area capacity per (dst, quarter)
DCHUNK = 224
NCHUNK = CN // DCHUNK   # 56
RPAD = CN + 6 * DCHUNK  # 13888
GCHUNK = RPAD // DCHUNK  # 62
OVW = RPAD - CN         # 1344 overflow cols
SLOTS = RPAD * K
SCHUNK = DCHUNK * K     # 1792
SENT = QREAL
ZCOL = RPAD - 1
CH = 15
BIG_NEG = -30000.0

LAST_STATS = {}


def _install_ntff_hook_module():
    if "antenv.axon_hooks" in sys.modules:
        return
    mod = types.ModuleType("antenv.axon_hooks")
    state = {"hook": None, "tried": False}

    def set_axon_ntff_profile_hook(hook):
        state["hook"] = hook

    def get_axon_ntff_profile_hook():
        if state["hook"] is None and not state["tried"]:
            state["tried"] = True
            try:
                from trn_agent_boot.trn_boot import _ntff_profile_via_ctypes

                state["hook"] = _ntff_profile_via_ctypes("/opt/axon/libaxon_pjrt.so")
            except Exception:
                state["hook"] = None
        return state["hook"]

    mod.set_axon_ntff_profile_hook = set_axon_ntff_profile_hook
    mod.get_axon_ntff_profile_hook = get_axon_ntff_profile_hook
    sys.modules["antenv.axon_hooks"] = mod


_install_ntff_hook_module()

import concourse.bass as bass
import concourse.mybir as mybir
import concourse.tile as tile
from concourse.bass_utils import run_bass_kernel_spmd
from concourse.library_overlay import lower_extended_insts
from concourse import library_config

F32 = mybir.dt.float32
BF16 = mybir.dt.bfloat16
I16 = mybir.dt.int16
ALU = mybir.AluOpType
ACT = mybir.ActivationFunctionType


def _split_wide_waits(nc):
    """Walrus here caps sync-wait commands per instruction; hoist excess waits
    onto preceding same-engine nofuse NOPs (engines execute in order)."""
    for fn in nc.m.functions:
        for bb in fn.blocks:
            new_insts = []
            for inst in bb.instructions:
                keep = 0 if isinstance(inst, mybir.InstDrain) else 1
                si = inst.sync_info
                if si is not None and si.on_wait is not None and len(si.on_wait) > keep:
                    waits = list(si.on_wait)
                    head, rest = (waits[:-keep], waits[-keep:]) if keep else (waits, [])
                    while head:
                        chunk, head = head[:1], head[1:]
                        nop = mybir.InstNoOp(name=f"I-{nc.next_id()}", ins=[], outs=[])
                        nop.engine = inst.engine
                        nop.bass_nofuse = True
                        nop.sync_info = mybir.SyncInfo(on_wait=chunk, on_update=[])
                        nc.register_instruction(nop, overwrite=True)
                        new_insts.append(nop)
                    inst.sync_info = mybir.SyncInfo(
                        on_wait=rest, on_update=list(si.on_update or [])
                    )
                new_insts.append(inst)
            bb.instructions.clear()
            for i in new_insts:
                bb.add_instruction(i)


def _run(nc, in_maps, trace=False):
    lower_extended_insts(nc)
    _split_wide_waits(nc)
    return run_bass_kernel_spmd(nc, in_maps, core_ids=list(range(NCORE)), trace=trace)


# ---------------------------------------------------------------- launch A
def _build_phase_a():
    nc = bass.Bass("TRN2")
    xT_d = nc.dram_tensor("xT", [IN_DIM, CN], F32, kind="ExternalInput")
    w1_d = nc.dram_tensor("w1", [IN_DIM, HEADS * HID], F32, kind="ExternalInput")
    w1t_d = nc.dram_tensor("w1t", [HEADS * HID, IN_DIM], F32, kind="ExternalInput")
    attw1_d = nc.dram_tensor("attw1", [HEADS * HID, 6], F32, kind="ExternalInput")
    tab_d = nc.dram_tensor("tab", [CH, CN], F32, kind="ExternalOutput")

    with tile.TileContext(nc) as tc:
        with (
            tc.tile_pool(name="const", bufs=1) as cpool,
            tc.tile_pool(name="io", bufs=3) as iopool,
            tc.tile_pool(name="ps", bufs=2, space="PSUM") as pspool,
        ):
            w1aug = cpool.tile([128, 2 * CH], F32)
            w1t = cpool.tile([HEADS * HID, IN_DIM], F32)
            attw1 = cpool.tile([HEADS * HID, 6], F32)
            nc.sync.dma_start(w1t[:], w1t_d[:])
            nc.sync.dma_start(attw1[:], attw1_d[:])
            for k in range(2):
                nc.sync.dma_start(
                    w1aug[:, CH * k:CH * k + 9], w1_d[128 * k:128 * (k + 1), :]
                )
                vps = pspool.tile([128, 6], F32, tag="vps")
                nc.tensor.matmul(
                    out=vps[:],
                    lhsT=w1t[:, 128 * k:128 * (k + 1)],
                    rhs=attw1[:],
                    start=True,
                    stop=True,
                )
                nc.vector.tensor_copy(out=w1aug[:, CH * k + 9:CH * k + 15], in_=vps[:])
            for c in range(NCHUNK):
                cols = slice(DCHUNK * c, DCHUNK * (c + 1))
                ps = pspool.tile([CH, DCHUNK], F32, tag="ps")
                for k in range(2):
                    xc = iopool.tile([128, DCHUNK], F32, tag="xc")
                    nc.sync.dma_start(xc[:], xT_d[128 * k:128 * (k + 1), cols])
                    nc.tensor.matmul(
                        out=ps[:],
                        lhsT=w1aug[:, CH * k:CH * (k + 1)],
                        rhs=xc[:],
                        start=(k == 0),
                        stop=(k == 1),
                    )
                ob = iopool.tile([CH, DCHUNK], F32, tag="ob")
                nc.vector.tensor_copy(out=ob[:], in_=ps[:])
                nc.sync.dma_start(tab_d[:, cols], ob[:])
    return nc


# ---------------------------------------------------------------- launch B/C
def _build_edge(final):
    nc = bass.Bass("TRN2")
    tab_d = nc.dram_tensor("tabf", [CH, NPAD], F32, kind="ExternalInput")
    idx_d = nc.dram_tensor("idxs", [128, SLOTS // 16], I16, kind="ExternalInput")
    ov_d = nc.dram_tensor(
        "ovidx", [128, NCHUNK * DCHUNK // 16], I16, kind="ExternalInput"
    )
    adrep_d = nc.dram_tensor("adrep", [128, RPAD], F32, kind="ExternalInput")
    w9map_d = nc.dram_tensor("w9map", [128, 128], F32, kind="ExternalInput")
    lhsn_d = nc.dram_tensor("lhsn", [128, 9], F32, kind="ExternalInput")
    lhsd_d = nc.dram_tensor("lhsd", [128, 9], F32, kind="ExternalInput")
    bias_d = nc.dram_tensor("biasv", [9, 1], F32, kind="ExternalInput")
    if final:
        meanw_d = nc.dram_tensor("meanw", [9, NCLS], F32, kind="ExternalInput")
        ones3_d = nc.dram_tensor("ones3", [NCLS, 1], F32, kind="ExternalInput")
        ones1_d = nc.dram_tensor("ones1", [1, NCLS], F32, kind="ExternalInput")
        out_d = nc.dram_tensor("outp", [NCLS, CN], F32, kind="ExternalOutput")
    else:
        w2aug_d = nc.dram_tensor("w2aug", [9, CH], F32, kind="ExternalInput")
        tab2_d = nc.dram_tensor("tab2", [CH, CN], F32, kind="ExternalOutput")

    with tile.TileContext(nc) as tc:
        with (
            tc.tile_pool(name="big", bufs=1) as bigpool,
            tc.tile_pool(name="gp", bufs=2) as gpool,
            tc.tile_pool(name="wp", bufs=2) as wpool,
            tc.tile_pool(name="pp", bufs=2) as ppool,
            tc.tile_pool(name="sm", bufs=2) as smpool,
            tc.tile_pool(name="pw", bufs=1, space="PSUM") as pwpool,
            tc.tile_pool(name="pn", bufs=2, space="PSUM") as pnpool,
        ):
            table = bigpool.tile([128, QN], F32)
            ovpre = bigpool.tile([128, NCHUNK * DCHUNK // 16], I16)
            adrep = bigpool.tile([128, RPAD], F32)
            w9map = bigpool.tile([128, 128], F32)
            lhsn = bigpool.tile([128, 9], F32)
            lhsd = bigpool.tile([128, 9], F32)
            biasv = bigpool.tile([9, 1], F32)
            ovbuf = bigpool.tile([128, OVW], F32)
            for g in range(8):
                q = g % 4
                nc.sync.dma_start(
                    table[16 * g:16 * g + CH, :], tab_d[:, QN * q:QN * (q + 1)]
                )
                # channel 15 of each group is never produced by the table DMA;
                # fill it with the ones row so gathers can't read uninit SBUF.
                nc.sync.dma_start(
                    table[16 * g + CH:16 * g + 16, :], tab_d[CH - 1:CH, QN * q:QN * (q + 1)]
                )
            nc.sync.dma_start(ovpre[:], ov_d[:])
            nc.sync.dma_start(adrep[:], adrep_d[:])
            nc.sync.dma_start(w9map[:], w9map_d[:])
            nc.sync.dma_start(lhsn[:], lhsn_d[:])
            nc.sync.dma_start(lhsd[:], lhsd_d[:])
            nc.sync.dma_start(biasv[:], bias_d[:])
            if final:
                meanw = bigpool.tile([9, NCLS], F32)
                ones3 = bigpool.tile([NCLS, 1], F32)
                ones1 = bigpool.tile([1, NCLS], F32)
                nc.sync.dma_start(meanw[:], meanw_d[:])
                nc.sync.dma_start(ones3[:], ones3_d[:])
                nc.sync.dma_start(ones1[:], ones1_d[:])
            else:
                w2aug = bigpool.tile([9, CH], F32)
                nc.sync.dma_start(w2aug[:], w2aug_d[:])

            tab_in = table[:].rearrange("p (n d) -> p n d", d=1)
            ov_in = ovbuf[:].rearrange("p (n d) -> p n d", d=1)
            nc.gpsimd.load_library(library_config.ap_gather)

            def do_chunk(c):
                """Gather + per-slot weights + weighted slot reduce for grid
                chunk c; returns the [128, DCHUNK] reduced partials tile."""
                scol = slice(SCHUNK // 16 * c, SCHUNK // 16 * (c + 1))
                dcol = slice(DCHUNK * c, DCHUNK * (c + 1))
                idxc = gpool.tile([128, SCHUNK // 16], I16, tag="idxc")
                nc.sync.dma_start(idxc[:], idx_d[:, scol])
                g_t = gpool.tile([128, SCHUNK], F32, tag="g")
                nc.gpsimd.ap_gather(
                    out_ap=g_t[:].rearrange("p (n d) -> p n d", d=1),
                    in_ap=tab_in,
                    idxs_ap=idxc[:],
                    channels=128,
                    num_elems=QN,
                    d=1,
                    num_idxs=SCHUNK,
                )
                # per-slot logits on all 128 rows (only 16g+9..11 meaningful)
                wadd = wpool.tile([128, SCHUNK], F32, tag="w")
                wadd3 = wadd[:].rearrange("p (n j) -> p n j", j=K)
                g3 = g_t[:].rearrange("p (n j) -> p n j", j=K)
                nc.vector.tensor_tensor(
                    out=wadd3, in0=g3,
                    in1=adrep[:, dcol].to_broadcast([128, DCHUNK, K]), op=ALU.add,
                )
                nc.vector.scalar_tensor_tensor(
                    out=wadd[:], in0=wadd[:], scalar=NEG, in1=wadd[:],
                    op0=ALU.mult, op1=ALU.max,
                )
                nc.scalar.activation(out=wadd[:], in_=wadd[:], func=ACT.Exp)
                # replicate head weights to channel rows with one 0/1 matmul
                w9ps = pwpool.tile([128, SCHUNK], F32, tag="w9")
                for s0 in range(0, SCHUNK, 512):
                    s1 = min(s0 + 512, SCHUNK)
                    nc.tensor.matmul(
                        out=w9ps[:, s0:s1], lhsT=w9map[:], rhs=wadd[:, s0:s1],
                        start=True, stop=True,
                    )
                nc.vector.tensor_tensor(
                    out=g_t[:], in0=g_t[:], in1=w9ps[:], op=ALU.mult
                )
                part = ppool.tile([128, DCHUNK], F32, tag="part")
                nc.vector.tensor_reduce(
                    out=part[:], in_=g3,
                    axis=mybir.AxisListType.X, op=ALU.add,
                )
                return part

            # overflow grid chunks first -> ovbuf
            for i, c in enumerate(range(NCHUNK, GCHUNK)):
                part = do_chunk(c)
                nc.vector.tensor_copy(
                    out=ovbuf[:, DCHUNK * i:DCHUNK * (i + 1)], in_=part[:]
                )

            # main chunks: fold overflow + combine + per-node math
            for c in range(NCHUNK):
                dcol = slice(DCHUNK * c, DCHUNK * (c + 1))
                part = do_chunk(c)
                fold = ppool.tile([128, DCHUNK], F32, tag="fold")
                nc.gpsimd.ap_gather(
                    out_ap=fold[:].rearrange("p (n d) -> p n d", d=1),
                    in_ap=ov_in,
                    idxs_ap=ovpre[:, DCHUNK // 16 * c:DCHUNK // 16 * (c + 1)],
                    channels=128,
                    num_elems=OVW,
                    d=1,
                    num_idxs=DCHUNK,
                )
                ndn = pnpool.tile([9, DCHUNK], F32, tag="ndn")
                ndd = pnpool.tile([9, DCHUNK], F32, tag="ndd")
                nc.tensor.matmul(
                    out=ndn[:], lhsT=lhsn[:], rhs=part[:], start=True, stop=False
                )
                nc.tensor.matmul(
                    out=ndn[:], lhsT=lhsn[:], rhs=fold[:], start=False, stop=True
                )
                nc.tensor.matmul(
                    out=ndd[:], lhsT=lhsd[:], rhs=part[:], start=True, stop=False
                )
                nc.tensor.matmul(
                    out=ndd[:], lhsT=lhsd[:], rhs=fold[:], start=False, stop=True
                )
                rden = smpool.tile([9, DCHUNK], F32, tag="rden")
                nc.vector.tensor_scalar_add(out=rden[:], in0=ndd[:], scalar1=EPS)
                nc.vector.reciprocal(out=rden[:], in_=rden[:])
                hagg = smpool.tile([9, DCHUNK], F32, tag="hagg")
                nc.vector.tensor_tensor(
                    out=hagg[:], in0=ndn[:], in1=rden[:], op=ALU.mult
                )
                nc.vector.tensor_scalar_add(
                    out=hagg[:], in0=hagg[:], scalar1=biasv[:]
                )
                if not final:
                    # elu = relu(x) + exp(min(x,0)) - 1
                    t1 = smpool.tile([9, DCHUNK], F32, tag="t1")
                    nc.vector.tensor_scalar_min(out=t1[:], in0=hagg[:], scalar1=0.0)
                    nc.scalar.activation(out=t1[:], in_=t1[:], func=ACT.Exp)
                    nc.vector.tensor_scalar_max(out=hagg[:], in0=hagg[:], scalar1=0.0)
                    nc.vector.tensor_tensor(
                        out=hagg[:], in0=hagg[:], in1=t1[:], op=ALU.add
                    )
                    nc.vector.tensor_scalar_add(out=hagg[:], in0=hagg[:], scalar1=-1.0)
                    t2 = pnpool.tile([CH, DCHUNK], F32, tag="ndn")
                    nc.tensor.matmul(
                        out=t2[:], lhsT=w2aug[:], rhs=hagg[:], start=True, stop=True
                    )
                    t2sb = smpool.tile([CH, DCHUNK], F32, tag="t2sb")
                    nc.vector.tensor_copy(out=t2sb[:], in_=t2[:])
                    nc.sync.dma_start(tab2_d[:, dcol], t2sb[:])
                else:
                    zps = pnpool.tile([NCLS, DCHUNK], F32, tag="ndn")
                    nc.tensor.matmul(
                        out=zps[:], lhsT=meanw[:], rhs=hagg[:], start=True, stop=True
                    )
                    z = smpool.tile([NCLS, DCHUNK], F32, tag="z")
                    nc.vector.tensor_scalar_add(
                        out=z[:], in0=zps[:], scalar1=biasv[0:NCLS, :]
                    )
                    ez = smpool.tile([NCLS, DCHUNK], F32, tag="ez")
                    nc.scalar.activation(out=ez[:], in_=z[:], func=ACT.Exp)
                    sps = pnpool.tile([1, DCHUNK], F32, tag="ndd")
                    nc.tensor.matmul(
                        out=sps[:], lhsT=ones3[:], rhs=ez[:], start=True, stop=True
                    )
                    s = smpool.tile([1, DCHUNK], F32, tag="s")
                    nc.scalar.activation(out=s[:], in_=sps[:], func=ACT.Ln)
                    l3ps = pnpool.tile([NCLS, DCHUNK], F32, tag="ndd")
                    nc.tensor.matmul(
                        out=l3ps[:], lhsT=ones1[:], rhs=s[:], start=True, stop=True
                    )
                    l3 = smpool.tile([NCLS, DCHUNK], F32, tag="l3")
                    nc.vector.tensor_copy(out=l3[:], in_=l3ps[:])
                    zm = smpool.tile([NCLS, DCHUNK], F32, tag="zm")
                    nc.vector.tensor_tensor(
                        out=zm[:], in0=z[:], in1=l3[:], op=ALU.subtract
                    )
                    nc.sync.dma_start(out_d[:, dcol], zm[:])
    return nc


# ---------------------------------------------------------------- host side
def _relabel(n):
    q = n // QREAL
    return q * QN + n % QREAL


def _wrap_chunked(stream, chunk):
    """[G, S] streams -> [16G, S//16] ap_gather idx layout, wrapped per chunk."""
    g, s = stream.shape
    nch = s // chunk
    w = stream.reshape(g, nch, chunk // 16, 16)
    w = w.transpose(0, 3, 1, 2)
    return np.ascontiguousarray(w.reshape(g * 16, s // 16))


def _pack_edges(src, dst):
    srcN = _relabel(src.astype(np.int64))
    dstN = _relabel(dst.astype(np.int64))
    core = dstN // CN
    dloc = dstN % CN
    q = srcN // QN
    sloc = (srcN % QN).astype(np.int16)

    key = (core * CN + dloc) * 4 + q
    order = np.argsort(key, kind="stable")
    ks = key[order]
    grp_start = np.r_[0, np.flatnonzero(np.diff(ks)) + 1]
    sizes = np.diff(np.r_[grp_start, len(ks)])
    rank = np.arange(len(ks)) - np.repeat(grp_start, sizes)

    co, dl, qo, sl = core[order], dloc[order], q[order], sloc[order]

    streams = np.full((NCORE, 8, SLOTS), SENT, dtype=np.int16)
    ovidx = np.full((NCORE, CN), ZCOL, dtype=np.int16)
    ovdst = [[] for _ in range(NCORE)]

    main = rank < CAP
    gmain = qo[main] + 4 * (rank[main] & 1)
    pos = dl[main] * K + (rank[main] >> 1)
    streams[co[main], gmain, pos] = sl[main]

    for i in np.flatnonzero(~main):
        c, d, qq, s_, r = co[i], dl[i], qo[i], sl[i], rank[i]
        if ovidx[c, d] == ZCOL:
            row = CN + len(ovdst[c])
            assert row < RPAD - 1, "overflow area exhausted"
            ovidx[c, d] = row
            ovdst[c].append(int(d))
        rr = r - CAP
        assert rr < CAP, "per-(dst,quarter) capacity exceeded"
        g = qq + 4 * (rr & 1)
        streams[c, g, int(ovidx[c, d]) * K + (rr >> 1)] = s_
    return streams, ovidx, ovdst


def kernel(x, edge_index, W1, att_src1, att_dst1, b1, W2, att_src2, att_dst2, b2):
    import os as _os
    import time as _time

    x = np.asarray(x, np.float32)
    W1 = np.asarray(W1, np.float32)
    W2 = np.asarray(W2, np.float32)
    b1v = np.asarray(b1, np.float32)
    b2v = np.asarray(b2, np.float32)

    loops = np.arange(N_NODES, dtype=np.int64)
    src = np.concatenate([np.asarray(edge_index[0], np.int64), loops])
    dst = np.concatenate([np.asarray(edge_index[1], np.int64), loops])
    streams, ovidx, ovdst = _pack_edges(src, dst)

    xP = np.zeros((NPAD, IN_DIM), np.float32)
    xP[_relabel(np.arange(N_NODES))] = x
    xT = np.ascontiguousarray(xP.T)

    def attw(att_s, att_d):
        a = np.zeros((HEADS * HID, 6), np.float32)
        for h in range(HEADS):
            for cc in range(3):
                a[3 * h + cc, h] = np.asarray(att_s, np.float32)[h, cc]
                a[3 * h + cc, 3 + h] = np.asarray(att_d, np.float32)[h, cc]
        return a

    attw1 = attw(att_src1, att_dst1)
    attw2 = attw(att_src2, att_dst2)

    # w9map: psum row 16g+3h+c (h channels) and 16g+12+h (ones channels)
    # both take the exp'd logit living on row 16g+9+h
    w9map = np.zeros((128, 128), np.float32)
    lhsn = np.zeros((128, 9), np.float32)
    lhsd = np.zeros((128, 9), np.float32)
    for g in range(8):
        for h in range(HEADS):
            for cc in range(3):
                w9map[16 * g + 9 + h, 16 * g + 3 * h + cc] = 1.0
                lhsd[16 * g + 12 + h, 3 * h + cc] = 1.0
            w9map[16 * g + 9 + h, 16 * g + 12 + h] = 1.0
        for j in range(9):
            lhsn[16 * g + j, j] = 1.0

    meanw = np.zeros((9, NCLS), np.float32)
    for h in range(HEADS):
        for cc in range(NCLS):
            meanw[3 * h + cc, cc] = 1.0 / 3.0
    ones3 = np.ones((NCLS, 1), np.float32)
    ones1 = np.ones((1, NCLS), np.float32)
    b1m = b1v.reshape(9, 1).copy()
    b2m = np.zeros((9, 1), np.float32)
    b2m[:NCLS, 0] = b2v
    w2aug = np.concatenate([W2, W2 @ attw2], axis=1).astype(np.float32)

    idx_wr = np.stack([_wrap_chunked(streams[c], SCHUNK) for c in range(NCORE)])
    # fold indices rebased into ovbuf coords (sentinel ZCOL -> OVW-1, a
    # guaranteed all-sentinel zero column)
    ov_wr = np.stack(
        [
            _wrap_chunked(
                np.repeat((ovidx[c] - CN).reshape(1, CN), 8, axis=0), DCHUNK
            )
            for c in range(NCORE)
        ]
    )

    def make_adrep(tab):
        """[128, RPAD] bf16 per core: rows 16g+9+h = a_dst[h] of the col's
        dst node (incl. overflow rows); all other rows 0."""
        out = []
        for c in range(NCORE):
            ad = np.zeros((3, RPAD), np.float32)
            ad[:, :CN] = tab[12:15, CN * c:CN * (c + 1)]
            for i, d in enumerate(ovdst[c]):
                ad[:, CN + i] = tab[12:15, CN * c + d]
            rep = np.zeros((128, RPAD), np.float32)
            for g in range(8):
                rep[16 * g + 9:16 * g + 12, :] = ad
            out.append(rep)
        return out

    trace = bool(int(_os.environ.get("KERNEL_TRACE", "0")))
    stats = {}
    t0 = _time.time()

    ncA = _build_phase_a()
    in_maps = [
        {
            "xT": np.ascontiguousarray(xT[:, CN * c:CN * (c + 1)]),
            "w1": W1,
            "w1t": np.ascontiguousarray(W1.T),
            "attw1": attw1,
        }
        for c in range(NCORE)
    ]
    resA = _run(ncA, in_maps, trace=trace)
    stats["A_ns"] = resA.exec_time_ns
    tab1 = np.concatenate([resA.results[c]["tab"] for c in range(NCORE)], axis=1)
    padmask = np.zeros(NPAD, bool)
    for qq in range(NQ):
        padmask[QN * qq + QREAL:QN * (qq + 1)] = True
    adreps = make_adrep(tab1)
    tab1[9:12, padmask] = BIG_NEG
    tab1[12:15, :] = 1.0

    ncB = _build_edge(final=False)
    in_maps = [
        {
            "tabf": tab1,
            "idxs": idx_wr[c],
            "ovidx": ov_wr[c],
            "adrep": adreps[c],
            "w9map": w9map,
            "lhsn": lhsn,
            "lhsd": lhsd,
            "biasv": b1m,
            "w2aug": w2aug,
        }
        for c in range(NCORE)
    ]
    resB = _run(ncB, in_maps, trace=trace)
    stats["B_ns"] = resB.exec_time_ns
    tab2 = np.concatenate([resB.results[c]["tab2"] for c in range(NCORE)], axis=1)
    adreps = make_adrep(tab2)
    tab2[9:12, padmask] = BIG_NEG
    tab2[12:15, :] = 1.0

    ncC = _build_edge(final=True)
    in_maps = [
        {
            "tabf": tab2,
            "idxs": idx_wr[c],
            "ovidx": ov_wr[c],
            "adrep": adreps[c],
            "w9map": w9map,
            "lhsn": lhsn,
            "lhsd": lhsd,
            "biasv": b2m,
            "meanw": meanw,
            "ones3": ones3,
            "ones1": ones1,
        }
        for c in range(NCORE)
    ]
    resC = _run(ncC, in_maps, trace=trace)
    stats["C_ns"] = resC.exec_time_ns
    outT = np.concatenate([resC.results[c]["outp"] for c in range(NCORE)], axis=1)
    stats["wall_s"] = _time.time() - t0

    out = outT.T[_relabel(np.arange(N_NODES))]
    LAST_STATS.clear()
    LAST_STATS.update(stats)
    return np.ascontiguousarray(out, dtype=np.float32)




# revision 26
# speedup vs baseline: 1.0593x; 1.0013x over previous
"""GAT (2-layer, 3-head) forward on 8 Trainium2 NeuronCores.

Sharding: nodes split 8 ways; each core owns 12544 padded destination nodes
and all their incoming edges (1D graph partition per the spec hint). A
channel-major node table (h | a_src | ones, 15 ch) is replicated into SBUF
as 4 quarters x 2 copies across the 8 GPSIMD 16-partition groups; per-edge
features stream out via ap_gather with per-group index streams laid out in
dst-canonical order with K=8 slots per (dst, group). All gather/fold index
streams and the a_dst broadcast table are preloaded into SBUF once. Per-edge
softmax weights are computed densely on all 128 partitions (broadcast add +
Lrelu/Exp on ACT), replicated to the h channels with a single 0/1 PE matmul
into PSUM, applied with one DVE multiply, and slot-window-reduced with an
in-place binary tree. Overflow rows are processed first into a small ovbuf
and folded back per chunk with a second tiny gather. Cross-group combine +
denominator replication uses lhsn/lhsd PE matmuls. Three NEFF launches:
(A) table build (x @ W1aug on PE), (B) edge layer 1 + layer-2 table build,
(C) edge layer 2 + head-mean + log_softmax. Tables are all-gathered between
launches through the host.
"""
import sys
import types

sys.path.insert(0, "/opt/trn_rl_repo")
import ml_dtypes
import numpy as np

N_NODES = 100000
IN_DIM = 256
HID = 3
HEADS = 3
NCLS = 3
NEG = 0.2
EPS = 1e-16

NQ = 4
QREAL = 25000
QN = 25088
NPAD = NQ * QN          # 100352
NCORE = 8
CN = NPAD // NCORE      # 12544
K = 7
CAP = 2 * K             # main-area capacity per (dst, quarter)
DCHUNK = 224
NCHUNK = CN // DCHUNK   # 56
RPAD = CN + 6 * DCHUNK  # 13888
GCHUNK = RPAD // DCHUNK  # 62
OVW = RPAD - CN         # 1344 overflow cols
SLOTS = RPAD * K
SCHUNK = DCHUNK * K     # 1792
SENT = QREAL
ZCOL = RPAD - 1
CH = 15
BIG_NEG = -30000.0

LAST_STATS = {}


def _install_ntff_hook_module():
    if "antenv.axon_hooks" in sys.modules:
        return
    mod = types.ModuleType("antenv.axon_hooks")
    state = {"hook": None, "tried": False}

    def set_axon_ntff_profile_hook(hook):
        state["hook"] = hook

    def get_axon_ntff_profile_hook():
        if state["hook"] is None and not state["tried"]:
            state["tried"] = True
            try:
                from trn_agent_boot.trn_boot import _ntff_profile_via_ctypes

                state["hook"] = _ntff_profile_via_ctypes("/opt/axon/libaxon_pjrt.so")
            except Exception:
                state["hook"] = None
        return state["hook"]

    mod.set_axon_ntff_profile_hook = set_axon_ntff_profile_hook
    mod.get_axon_ntff_profile_hook = get_axon_ntff_profile_hook
    sys.modules["antenv.axon_hooks"] = mod


_install_ntff_hook_module()

import concourse.bass as bass
import concourse.mybir as mybir
import concourse.tile as tile
from concourse.bass_utils import run_bass_kernel_spmd
from concourse.library_overlay import lower_extended_insts
from concourse import library_config

F32 = mybir.dt.float32
BF16 = mybir.dt.bfloat16
I16 = mybir.dt.int16
ALU = mybir.AluOpType
ACT = mybir.ActivationFunctionType


def _split_wide_waits(nc):
    """Walrus here caps sync-wait commands per instruction; hoist excess waits
    onto preceding same-engine nofuse NOPs (engines execute in order)."""
    for fn in nc.m.functions:
        for bb in fn.blocks:
            new_insts = []
            for inst in bb.instructions:
                keep = 0 if isinstance(inst, mybir.InstDrain) else 1
                si = inst.sync_info
                if si is not None and si.on_wait is not None and len(si.on_wait) > keep:
                    waits = list(si.on_wait)
                    head, rest = (waits[:-keep], waits[-keep:]) if keep else (waits, [])
                    while head:
                        chunk, head = head[:1], head[1:]
                        nop = mybir.InstNoOp(name=f"I-{nc.next_id()}", ins=[], outs=[])
                        nop.engine = inst.engine
                        nop.bass_nofuse = True
                        nop.sync_info = mybir.SyncInfo(on_wait=chunk, on_update=[])
                        nc.register_instruction(nop, overwrite=True)
                        new_insts.append(nop)
                    inst.sync_info = mybir.SyncInfo(
                        on_wait=rest, on_update=list(si.on_update or [])
                    )
                new_insts.append(inst)
            bb.instructions.clear()
            for i in new_insts:
                bb.add_instruction(i)


def _run(nc, in_maps, trace=False):
    lower_extended_insts(nc)
    _split_wide_waits(nc)
    return run_bass_kernel_spmd(nc, in_maps, core_ids=list(range(NCORE)), trace=trace)


# ---------------------------------------------------------------- launch A
def _build_phase_a():
    nc = bass.Bass("TRN2")
    xT_d = nc.dram_tensor("xT", [IN_DIM, CN], F32, kind="ExternalInput")
    w1_d = nc.dram_tensor("w1", [IN_DIM, HEADS * HID], F32, kind="ExternalInput")
    w1t_d = nc.dram_tensor("w1t", [HEADS * HID, IN_DIM], F32, kind="ExternalInput")
    attw1_d = nc.dram_tensor("attw1", [HEADS * HID, 6], F32, kind="ExternalInput")
    tab_d = nc.dram_tensor("tab", [CH, CN], F32, kind="ExternalOutput")

    with tile.TileContext(nc) as tc:
        with (
            tc.tile_pool(name="const", bufs=1) as cpool,
            tc.tile_pool(name="io", bufs=6) as iopool,
            tc.tile_pool(name="ps", bufs=4, space="PSUM") as pspool,
        ):
            w1aug = cpool.tile([128, 2 * CH], F32)
            w1t = cpool.tile([HEADS * HID, IN_DIM], F32)
            attw1 = cpool.tile([HEADS * HID, 6], F32)
            nc.sync.dma_start(w1t[:], w1t_d[:])
            nc.sync.dma_start(attw1[:], attw1_d[:])
            for k in range(2):
                nc.sync.dma_start(
                    w1aug[:, CH * k:CH * k + 9], w1_d[128 * k:128 * (k + 1), :]
                )
                vps = pspool.tile([128, 6], F32, tag="vps")
                nc.tensor.matmul(
                    out=vps[:],
                    lhsT=w1t[:, 128 * k:128 * (k + 1)],
                    rhs=attw1[:],
                    start=True,
                    stop=True,
                )
                nc.vector.tensor_copy(out=w1aug[:, CH * k + 9:CH * k + 15], in_=vps[:])
            for c in range(NCHUNK):
                cols = slice(DCHUNK * c, DCHUNK * (c + 1))
                ps = pspool.tile([CH, DCHUNK], F32, tag="ps")
                for k in range(2):
                    xc = iopool.tile([128, DCHUNK], F32, tag="xc")
                    nc.sync.dma_start(xc[:], xT_d[128 * k:128 * (k + 1), cols])
                    nc.tensor.matmul(
                        out=ps[:],
                        lhsT=w1aug[:, CH * k:CH * (k + 1)],
                        rhs=xc[:],
                        start=(k == 0),
                        stop=(k == 1),
                    )
                ob = iopool.tile([CH, DCHUNK], F32, tag="ob")
                nc.vector.tensor_copy(out=ob[:], in_=ps[:])
                nc.sync.dma_start(tab_d[:, cols], ob[:])
    return nc


# ---------------------------------------------------------------- launch B/C
def _build_edge(final):
    nc = bass.Bass("TRN2")
    tab_d = nc.dram_tensor("tabf", [CH, NPAD], F32, kind="ExternalInput")
    idx_d = nc.dram_tensor("idxs", [128, SLOTS // 16], I16, kind="ExternalInput")
    ov_d = nc.dram_tensor(
        "ovidx", [128, NCHUNK * DCHUNK // 16], I16, kind="ExternalInput"
    )
    adrep_d = nc.dram_tensor("adrep", [128, RPAD], F32, kind="ExternalInput")
    w9map_d = nc.dram_tensor("w9map", [128, 128], F32, kind="ExternalInput")
    lhsn_d = nc.dram_tensor("lhsn", [128, 9], F32, kind="ExternalInput")
    lhsd_d = nc.dram_tensor("lhsd", [128, 9], F32, kind="ExternalInput")
    bias_d = nc.dram_tensor("biasv", [9, 1], F32, kind="ExternalInput")
    if final:
        meanw_d = nc.dram_tensor("meanw", [9, NCLS], F32, kind="ExternalInput")
        ones3_d = nc.dram_tensor("ones3", [NCLS, 1], F32, kind="ExternalInput")
        ones1_d = nc.dram_tensor("ones1", [1, NCLS], F32, kind="ExternalInput")
        out_d = nc.dram_tensor("outp", [NCLS, CN], F32, kind="ExternalOutput")
    else:
        w2aug_d = nc.dram_tensor("w2aug", [9, CH], F32, kind="ExternalInput")
        tab2_d = nc.dram_tensor("tab2", [CH, CN], F32, kind="ExternalOutput")

    with tile.TileContext(nc) as tc:
        with (
            tc.tile_pool(name="big", bufs=1) as bigpool,
            tc.tile_pool(name="gp", bufs=2) as gpool,
            tc.tile_pool(name="wp", bufs=2) as wpool,
            tc.tile_pool(name="pp", bufs=2) as ppool,
            tc.tile_pool(name="sm", bufs=2) as smpool,
            tc.tile_pool(name="pw", bufs=1, space="PSUM") as pwpool,
            tc.tile_pool(name="pn", bufs=2, space="PSUM") as pnpool,
        ):
            table = bigpool.tile([128, QN], F32)
            ovpre = bigpool.tile([128, NCHUNK * DCHUNK // 16], I16)
            adrep = bigpool.tile([128, RPAD], F32)
            w9map = bigpool.tile([128, 128], F32)
            lhsn = bigpool.tile([128, 9], F32)
            lhsd = bigpool.tile([128, 9], F32)
            biasv = bigpool.tile([9, 1], F32)
            ovbuf = bigpool.tile([128, OVW], F32)
            for g in range(8):
                q = g % 4
                nc.sync.dma_start(
                    table[16 * g:16 * g + CH, :], tab_d[:, QN * q:QN * (q + 1)]
                )
                # channel 15 of each group is never produced by the table DMA;
                # fill it with the ones row so gathers can't read uninit SBUF.
                nc.sync.dma_start(
                    table[16 * g + CH:16 * g + 16, :], tab_d[CH - 1:CH, QN * q:QN * (q + 1)]
                )
            nc.sync.dma_start(ovpre[:], ov_d[:])
            nc.sync.dma_start(adrep[:], adrep_d[:])
            nc.sync.dma_start(w9map[:], w9map_d[:])
            nc.sync.dma_start(lhsn[:], lhsn_d[:])
            nc.sync.dma_start(lhsd[:], lhsd_d[:])
            nc.sync.dma_start(biasv[:], bias_d[:])
            if final:
                meanw = bigpool.tile([9, NCLS], F32)
                ones3 = bigpool.tile([NCLS, 1], F32)
                ones1 = bigpool.tile([1, NCLS], F32)
                nc.sync.dma_start(meanw[:], meanw_d[:])
                nc.sync.dma_start(ones3[:], ones3_d[:])
                nc.sync.dma_start(ones1[:], ones1_d[:])
            else:
                w2aug = bigpool.tile([9, CH], F32)
                nc.sync.dma_start(w2aug[:], w2aug_d[:])

            tab_in = table[:].rearrange("p (n d) -> p n d", d=1)
            ov_in = ovbuf[:].rearrange("p (n d) -> p n d", d=1)
            nc.gpsimd.load_library(library_config.ap_gather)

            def do_chunk(c):
                """Gather + per-slot weights + weighted slot reduce for grid
                chunk c; returns the [128, DCHUNK] reduced partials tile."""
                scol = slice(SCHUNK // 16 * c, SCHUNK // 16 * (c + 1))
                dcol = slice(DCHUNK * c, DCHUNK * (c + 1))
                idxc = gpool.tile([128, SCHUNK // 16], I16, tag="idxc")
                nc.sync.dma_start(idxc[:], idx_d[:, scol])
                g_t = gpool.tile([128, SCHUNK], F32, tag="g")
                nc.gpsimd.ap_gather(
                    out_ap=g_t[:].rearrange("p (n d) -> p n d", d=1),
                    in_ap=tab_in,
                    idxs_ap=idxc[:],
                    channels=128,
                    num_elems=QN,
                    d=1,
                    num_idxs=SCHUNK,
                )
                # per-slot logits on all 128 rows (only 16g+9..11 meaningful)
                wadd = wpool.tile([128, SCHUNK], F32, tag="w")
                wadd3 = wadd[:].rearrange("p (n j) -> p n j", j=K)
                g3 = g_t[:].rearrange("p (n j) -> p n j", j=K)
                nc.vector.tensor_tensor(
                    out=wadd3, in0=g3,
                    in1=adrep[:, dcol].to_broadcast([128, DCHUNK, K]), op=ALU.add,
                )
                nc.vector.scalar_tensor_tensor(
                    out=wadd[:], in0=wadd[:], scalar=NEG, in1=wadd[:],
                    op0=ALU.mult, op1=ALU.max,
                )
                nc.scalar.activation(out=wadd[:], in_=wadd[:], func=ACT.Exp)
                # replicate head weights to channel rows with one 0/1 matmul
                w9ps = pwpool.tile([128, SCHUNK], F32, tag="w9")
                for s0 in range(0, SCHUNK, 512):
                    s1 = min(s0 + 512, SCHUNK)
                    nc.tensor.matmul(
                        out=w9ps[:, s0:s1], lhsT=w9map[:], rhs=wadd[:, s0:s1],
                        start=True, stop=True,
                    )
                nc.vector.tensor_tensor(
                    out=g_t[:], in0=g_t[:], in1=w9ps[:], op=ALU.mult
                )
                part = ppool.tile([128, DCHUNK], F32, tag="part")
                nc.vector.tensor_reduce(
                    out=part[:], in_=g3,
                    axis=mybir.AxisListType.X, op=ALU.add,
                )
                return part

            # overflow grid chunks first -> ovbuf
            for i, c in enumerate(range(NCHUNK, GCHUNK)):
                part = do_chunk(c)
                nc.vector.tensor_copy(
                    out=ovbuf[:, DCHUNK * i:DCHUNK * (i + 1)], in_=part[:]
                )

            # main chunks: fold overflow + combine + per-node math
            for c in range(NCHUNK):
                dcol = slice(DCHUNK * c, DCHUNK * (c + 1))
                part = do_chunk(c)
                fold = ppool.tile([128, DCHUNK], F32, tag="fold")
                nc.gpsimd.ap_gather(
                    out_ap=fold[:].rearrange("p (n d) -> p n d", d=1),
                    in_ap=ov_in,
                    idxs_ap=ovpre[:, DCHUNK // 16 * c:DCHUNK // 16 * (c + 1)],
                    channels=128,
                    num_elems=OVW,
                    d=1,
                    num_idxs=DCHUNK,
                )
                ndn = pnpool.tile([9, DCHUNK], F32, tag="ndn")
                ndd = pnpool.tile([9, DCHUNK], F32, tag="ndd")
                nc.tensor.matmul(
                    out=ndn[:], lhsT=lhsn[:], rhs=part[:], start=True, stop=False
                )
                nc.tensor.matmul(
                    out=ndn[:], lhsT=lhsn[:], rhs=fold[:], start=False, stop=True
                )
                nc.tensor.matmul(
                    out=ndd[:], lhsT=lhsd[:], rhs=part[:], start=True, stop=False
                )
                nc.tensor.matmul(
                    out=ndd[:], lhsT=lhsd[:], rhs=fold[:], start=False, stop=True
                )
                rden = smpool.tile([9, DCHUNK], F32, tag="rden")
                nc.vector.tensor_scalar_add(out=rden[:], in0=ndd[:], scalar1=EPS)
                nc.vector.reciprocal(out=rden[:], in_=rden[:])
                hagg = smpool.tile([9, DCHUNK], F32, tag="hagg")
                nc.vector.tensor_tensor(
                    out=hagg[:], in0=ndn[:], in1=rden[:], op=ALU.mult
                )
                nc.vector.tensor_scalar_add(
                    out=hagg[:], in0=hagg[:], scalar1=biasv[:]
                )
                if not final:
                    # elu = relu(x) + exp(min(x,0)) - 1
                    t1 = smpool.tile([9, DCHUNK], F32, tag="t1")
                    nc.vector.tensor_scalar_min(out=t1[:], in0=hagg[:], scalar1=0.0)
                    nc.scalar.activation(out=t1[:], in_=t1[:], func=ACT.Exp)
                    nc.vector.tensor_scalar_max(out=hagg[:], in0=hagg[:], scalar1=0.0)
                    nc.vector.tensor_tensor(
                        out=hagg[:], in0=hagg[:], in1=t1[:], op=ALU.add
                    )
                    nc.vector.tensor_scalar_add(out=hagg[:], in0=hagg[:], scalar1=-1.0)
                    t2 = pnpool.tile([CH, DCHUNK], F32, tag="ndn")
                    nc.tensor.matmul(
                        out=t2[:], lhsT=w2aug[:], rhs=hagg[:], start=True, stop=True
                    )
                    t2sb = smpool.tile([CH, DCHUNK], F32, tag="t2sb")
                    nc.vector.tensor_copy(out=t2sb[:], in_=t2[:])
                    nc.sync.dma_start(tab2_d[:, dcol], t2sb[:])
                else:
                    zps = pnpool.tile([NCLS, DCHUNK], F32, tag="ndn")
                    nc.tensor.matmul(
                        out=zps[:], lhsT=meanw[:], rhs=hagg[:], start=True, stop=True
                    )
                    z = smpool.tile([NCLS, DCHUNK], F32, tag="z")
                    nc.vector.tensor_scalar_add(
                        out=z[:], in0=zps[:], scalar1=biasv[0:NCLS, :]
                    )
                    ez = smpool.tile([NCLS, DCHUNK], F32, tag="ez")
                    nc.scalar.activation(out=ez[:], in_=z[:], func=ACT.Exp)
                    sps = pnpool.tile([1, DCHUNK], F32, tag="ndd")
                    nc.tensor.matmul(
                        out=sps[:], lhsT=ones3[:], rhs=ez[:], start=True, stop=True
                    )
                    s = smpool.tile([1, DCHUNK], F32, tag="s")
                    nc.scalar.activation(out=s[:], in_=sps[:], func=ACT.Ln)
                    l3ps = pnpool.tile([NCLS, DCHUNK], F32, tag="ndd")
                    nc.tensor.matmul(
                        out=l3ps[:], lhsT=ones1[:], rhs=s[:], start=True, stop=True
                    )
                    l3 = smpool.tile([NCLS, DCHUNK], F32, tag="l3")
                    nc.vector.tensor_copy(out=l3[:], in_=l3ps[:])
                    zm = smpool.tile([NCLS, DCHUNK], F32, tag="zm")
                    nc.vector.tensor_tensor(
                        out=zm[:], in0=z[:], in1=l3[:], op=ALU.subtract
                    )
                    nc.sync.dma_start(out_d[:, dcol], zm[:])
    return nc


# ---------------------------------------------------------------- host side
def _relabel(n):
    q = n // QREAL
    return q * QN + n % QREAL


def _wrap_chunked(stream, chunk):
    """[G, S] streams -> [16G, S//16] ap_gather idx layout, wrapped per chunk."""
    g, s = stream.shape
    nch = s // chunk
    w = stream.reshape(g, nch, chunk // 16, 16)
    w = w.transpose(0, 3, 1, 2)
    return np.ascontiguousarray(w.reshape(g * 16, s // 16))


def _pack_edges(src, dst):
    srcN = _relabel(src.astype(np.int64))
    dstN = _relabel(dst.astype(np.int64))
    core = dstN // CN
    dloc = dstN % CN
    q = srcN // QN
    sloc = (srcN % QN).astype(np.int16)

    key = (core * CN + dloc) * 4 + q
    order = np.argsort(key, kind="stable")
    ks = key[order]
    grp_start = np.r_[0, np.flatnonzero(np.diff(ks)) + 1]
    sizes = np.diff(np.r_[grp_start, len(ks)])
    rank = np.arange(len(ks)) - np.repeat(grp_start, sizes)

    co, dl, qo, sl = core[order], dloc[order], q[order], sloc[order]

    streams = np.full((NCORE, 8, SLOTS), SENT, dtype=np.int16)
    ovidx = np.full((NCORE, CN), ZCOL, dtype=np.int16)
    ovdst = [[] for _ in range(NCORE)]

    # main-area edges: prefer the copy matching the src half-quarter (locality
    # for the gather's reads), spilling to the other copy when one side is full
    mainm = rank < CAP
    gid = (core * CN + dloc) * 4 + q
    sub = np.flatnonzero(mainm)
    gidm = gid[order][sub]
    halfm = sl[sub].astype(np.int64) >= QN // 2
    key2 = gidm * 2 + halfm
    o2 = np.argsort(key2, kind="stable")
    k2s = key2[o2]
    st2 = np.r_[0, np.flatnonzero(np.diff(k2s)) + 1]
    sz2 = np.diff(np.r_[st2, len(k2s)])
    r2 = np.arange(len(k2s)) - np.repeat(st2, sz2)
    cnt = np.bincount(key2, minlength=int(gid.max()) * 2 + 2)
    gide = gidm[o2]
    lowm = ~halfm[o2]
    cl = np.minimum(cnt[gide * 2], K)
    ch_ = np.minimum(cnt[gide * 2 + 1], K)
    inA = np.where(lowm, r2 < K, r2 >= K)
    slot = np.where(
        lowm,
        np.where(r2 < K, r2, ch_ + (r2 - K)),
        np.where(r2 < K, r2, cl + (r2 - K)),
    )
    assert slot.max() < K
    qe = gide % 4
    dle = (gide // 4) % CN
    coe = gide // (4 * CN)
    streams[coe, qe + np.where(inA, 0, 4), dle * K + slot] = sl[sub][o2]

    for i in np.flatnonzero(~mainm):
        c, d, qq, s_, r = co[i], dl[i], qo[i], sl[i], rank[i]
        if ovidx[c, d] == ZCOL:
            row = CN + len(ovdst[c])
            assert row < RPAD - 1, "overflow area exhausted"
            ovidx[c, d] = row
            ovdst[c].append(int(d))
        rr = r - CAP
        assert rr < CAP, "per-(dst,quarter) capacity exceeded"
        g = qq + 4 * (rr & 1)
        streams[c, g, int(ovidx[c, d]) * K + (rr >> 1)] = s_
    return streams, ovidx, ovdst


def kernel(x, edge_index, W1, att_src1, att_dst1, b1, W2, att_src2, att_dst2, b2):
    import os as _os
    import time as _time

    x = np.asarray(x, np.float32)
    W1 = np.asarray(W1, np.float32)
    W2 = np.asarray(W2, np.float32)
    b1v = np.asarray(b1, np.float32)
    b2v = np.asarray(b2, np.float32)

    loops = np.arange(N_NODES, dtype=np.int64)
    src = np.concatenate([np.asarray(edge_index[0], np.int64), loops])
    dst = np.concatenate([np.asarray(edge_index[1], np.int64), loops])
    streams, ovidx, ovdst = _pack_edges(src, dst)

    xP = np.zeros((NPAD, IN_DIM), np.float32)
    xP[_relabel(np.arange(N_NODES))] = x
    xT = np.ascontiguousarray(xP.T)

    def attw(att_s, att_d):
        a = np.zeros((HEADS * HID, 6), np.float32)
        for h in range(HEADS):
            for cc in range(3):
                a[3 * h + cc, h] = np.asarray(att_s, np.float32)[h, cc]
                a[3 * h + cc, 3 + h] = np.asarray(att_d, np.float32)[h, cc]
        return a

    attw1 = attw(att_src1, att_dst1)
    attw2 = attw(att_src2, att_dst2)

    # w9map: psum row 16g+3h+c (h channels) and 16g+12+h (ones channels)
    # both take the exp'd logit living on row 16g+9+h
    w9map = np.zeros((128, 128), np.float32)
    lhsn = np.zeros((128, 9), np.float32)
    lhsd = np.zeros((128, 9), np.float32)
    for g in range(8):
        for h in range(HEADS):
            for cc in range(3):
                w9map[16 * g + 9 + h, 16 * g + 3 * h + cc] = 1.0
                lhsd[16 * g + 12 + h, 3 * h + cc] = 1.0
            w9map[16 * g + 9 + h, 16 * g + 12 + h] = 1.0
        for j in range(9):
            lhsn[16 * g + j, j] = 1.0

    meanw = np.zeros((9, NCLS), np.float32)
    for h in range(HEADS):
        for cc in range(NCLS):
            meanw[3 * h + cc, cc] = 1.0 / 3.0
    ones3 = np.ones((NCLS, 1), np.float32)
    ones1 = np.ones((1, NCLS), np.float32)
    b1m = b1v.reshape(9, 1).copy()
    b2m = np.zeros((9, 1), np.float32)
    b2m[:NCLS, 0] = b2v
    w2aug = np.concatenate([W2, W2 @ attw2], axis=1).astype(np.float32)

    idx_wr = np.stack([_wrap_chunked(streams[c], SCHUNK) for c in range(NCORE)])
    # fold indices rebased into ovbuf coords (sentinel ZCOL -> OVW-1, a
    # guaranteed all-sentinel zero column)
    ov_wr = np.stack(
        [
            _wrap_chunked(
                np.repeat((ovidx[c] - CN).reshape(1, CN), 8, axis=0), DCHUNK
            )
            for c in range(NCORE)
        ]
    )

    def make_adrep(tab):
        """[128, RPAD] bf16 per core: rows 16g+9+h = a_dst[h] of the col's
        dst node (incl. overflow rows); all other rows 0."""
        out = []
        for c in range(NCORE):
            ad = np.zeros((3, RPAD), np.float32)
            ad[:, :CN] = tab[12:15, CN * c:CN * (c + 1)]
            for i, d in enumerate(ovdst[c]):
                ad[:, CN + i] = tab[12:15, CN * c + d]
            rep = np.zeros((128, RPAD), np.float32)
            for g in range(8):
                rep[16 * g + 9:16 * g + 12, :] = ad
            out.append(rep)
        return out

    trace = bool(int(_os.environ.get("KERNEL_TRACE", "0")))
    stats = {}
    t0 = _time.time()

    ncA = _build_phase_a()
    in_maps = [
        {
            "xT": np.ascontiguousarray(xT[:, CN * c:CN * (c + 1)]),
            "w1": W1,
            "w1t": np.ascontiguousarray(W1.T),
            "attw1": attw1,
        }
        for c in range(NCORE)
    ]
    resA = _run(ncA, in_maps, trace=trace)
    stats["A_ns"] = resA.exec_time_ns
    tab1 = np.concatenate([resA.results[c]["tab"] for c in range(NCORE)], axis=1)
    padmask = np.zeros(NPAD, bool)
    for qq in range(NQ):
        padmask[QN * qq + QREAL:QN * (qq + 1)] = True
    adreps = make_adrep(tab1)
    tab1[9:12, padmask] = BIG_NEG
    tab1[12:15, :] = 1.0

    ncB = _build_edge(final=False)
    in_maps = [
        {
            "tabf": tab1,
            "idxs": idx_wr[c],
            "ovidx": ov_wr[c],
            "adrep": adreps[c],
            "w9map": w9map,
            "lhsn": lhsn,
            "lhsd": lhsd,
            "biasv": b1m,
            "w2aug": w2aug,
        }
        for c in range(NCORE)
    ]
    resB = _run(ncB, in_maps, trace=trace)
    stats["B_ns"] = resB.exec_time_ns
    tab2 = np.concatenate([resB.results[c]["tab2"] for c in range(NCORE)], axis=1)
    adreps = make_adrep(tab2)
    tab2[9:12, padmask] = BIG_NEG
    tab2[12:15, :] = 1.0

    ncC = _build_edge(final=True)
    in_maps = [
        {
            "tabf": tab2,
            "idxs": idx_wr[c],
            "ovidx": ov_wr[c],
            "adrep": adreps[c],
            "w9map": w9map,
            "lhsn": lhsn,
            "lhsd": lhsd,
            "biasv": b2m,
            "meanw": meanw,
            "ones3": ones3,
            "ones1": ones1,
        }
        for c in range(NCORE)
    ]
    resC = _run(ncC, in_maps, trace=trace)
    stats["C_ns"] = resC.exec_time_ns
    outT = np.concatenate([resC.results[c]["outp"] for c in range(NCORE)], axis=1)
    stats["wall_s"] = _time.time() - t0

    out = outT.T[_relabel(np.arange(N_NODES))]
    LAST_STATS.clear()
    LAST_STATS.update(stats)
    return np.ascontiguousarray(out, dtype=np.float32)


